# revision 1
# baseline (speedup 1.0000x reference)
"""Multi-head cross-attention (B=2, S=2048, D=1024, H=16) on 8 trn2 cores.

Sharding: core c -> (batch b = c//4, head-group g = c%4, 4 heads per group).
Tensor-parallel heads: wq/wk/wv column-sliced, wo row-sliced; partial outputs
summed on host.  Key-mask compaction on host: only unmasked keys are shipped
(padded to K_PAD), since masked keys contribute exactly zero after softmax.

Matmul operands are fp16 (PE streams fp32 at half rate); PSUM accumulation and
the softmax-normalization chain stay fp32.

Algebraic folds (exact): bk shifts every logit of a query by a constant ->
softmax-invariant -> dropped.  bv adds sum(weights)*bv = bv to every attention
output -> (bv @ wo + bo) added on host.
"""

import math
import sys
import types
from contextlib import ExitStack

import numpy as np

# --- shim antenv.axon_hooks so trace=True works under axon -----------------
if "antenv.axon_hooks" not in sys.modules:
    _mod = types.ModuleType("antenv.axon_hooks")
    _hook_box = [None]
    _mod.set_axon_ntff_profile_hook = lambda h: _hook_box.__setitem__(0, h)
    _mod.get_axon_ntff_profile_hook = lambda: _hook_box[0]
    sys.modules["antenv.axon_hooks"] = _mod
    try:
        import antenv

        antenv.axon_hooks = _mod
        from trn_agent_boot.trn_boot import _ntff_profile_via_ctypes

        _mod.set_axon_ntff_profile_hook(
            _ntff_profile_via_ctypes("/opt/axon/libaxon_pjrt.so")
        )
    except Exception:
        pass

import concourse.bass as bass
import concourse.mybir as mybir
import concourse.tile as tile
from concourse.bass_utils import run_bass_kernel_spmd
from concourse.vector_clock import ScopedClock

# --- patch Tile tail drain: this walrus build rejects CTRL insts with >1-2
# sem waits ("Too many sync wait commands").  Split the tail drain's waits
# into one drain per outstanding proc tick.
def _drain_and_barrier_split(self, tick_clock, wait_clock):
    nc = self.nc
    g = ScopedClock({None: tick_clock.global_clock})
    for scope, vc in g.items():
        for proc in range(len(vc)):
            t = vc[proc]
            if t > 0:
                sc = ScopedClock()
                sc.require_at_least(scope, proc, t)
                d = nc.sync.drain()
                wait_clock.add_sem_waits(d.ins, sc)
    nc.all_engine_barrier()
    assert self.sems is not None
    popped = nc._tile_sem_poison_stack.pop()
    assert popped is self._sem_poison
    nc.clear_and_free_semaphores(list(self.sems.allocated().values()))
    nc.all_engine_barrier()


tile.TileContext._drain_and_barrier = _drain_and_barrier_split

# This walrus build tolerates only 1 sync wait per instruction.  Hoist excess
# waits onto preceding EVENT_SEMAPHORE nops (the native wait_ge carrier).
_MAX_WAITS = 1
_orig_lower = tile.TileContext._lower_ordered_insts


def _is_self_wait(inst, w):
    # A ge-wait on the instruction's own engine sem is transitively implied
    # by in-order execution (Tile's vector clock is not transitively minimal).
    if w.wait_mode != "sem-ge-imm" or not w.ant_name:
        return False
    eng = str(inst.engine).split(".")[-1]
    return w.ant_name.startswith(eng + "_")


def _lower_split_waits(self, ordered):
    nc = self.nc
    for bb_name, insts in ordered.items():
        out = []
        for inst in insts:
            si = inst.sync_info
            if si is not None and si.on_wait:
                waits = [w for w in si.on_wait if not _is_self_wait(inst, w)]
                if len(waits) != len(si.on_wait) or len(waits) > _MAX_WAITS:
                    excess, keep = waits[:-_MAX_WAITS], waits[-_MAX_WAITS:]
                    for w in excess:
                        d = mybir.InstEventSemaphore(
                            name=nc.get_next_instruction_name(), ins=[], outs=[]
                        )
                        d.engine = inst.engine
                        d.sync_info = mybir.SyncInfo(on_wait=[w], on_update=[])
                        out.append(d)
                    inst.sync_info = mybir.SyncInfo(
                        on_wait=keep, on_update=list(si.on_update)
                    )
            out.append(inst)
        insts[:] = out
    return _orig_lower(self, ordered)


tile.TileContext._lower_ordered_insts = _lower_split_waits

F32 = mybir.dt.float32
F16 = mybir.dt.float16
B, S, D, H = 2, 2048, 1024, 16
DH = 64
G = 4  # head-groups == cores per batch
CD = D // G  # 256 head dims per core (4 heads)
N_CORES = 8
NEG = -1.0e30
K_PAD_LADDER = (1152, 1536, 2048)
Exp = mybir.ActivationFunctionType.Exp
Identity = mybir.ActivationFunctionType.Identity


def _build(k_pad: int) -> bass.Bass:
    kt_tiles = k_pad // 128
    kchunks = []
    off = 0
    while off < k_pad:
        c = min(512, k_pad - off)
        kchunks.append((off, c))
        off += c

    nc = bass.Bass()
    xT = nc.dram_tensor("xT", [D, S], F16, kind="ExternalInput")
    memT = nc.dram_tensor("memT", [D, k_pad], F16, kind="ExternalInput")
    wq_d = nc.dram_tensor("wq", [D, CD], F16, kind="ExternalInput")
    wk_d = nc.dram_tensor("wk", [D, CD], F16, kind="ExternalInput")
    wv_d = nc.dram_tensor("wv", [D, CD], F16, kind="ExternalInput")
    bq_d = nc.dram_tensor("bq", [128, 2], F32, kind="ExternalInput")
    wo_d = nc.dram_tensor("wo", [CD, D], F16, kind="ExternalInput")
    mb_d = nc.dram_tensor("maskb", [128, kt_tiles], F32, kind="ExternalInput")
    out_d = nc.dram_tensor("out", [S, D], F32, kind="ExternalOutput")

    with tile.TileContext(nc) as tc, ExitStack() as ctx:
        consts = ctx.enter_context(tc.tile_pool(name="consts", bufs=1))
        bigin = ctx.enter_context(tc.tile_pool(name="bigin", bufs=1))
        wpool = ctx.enter_context(tc.tile_pool(name="wp", bufs=1))
        qkv = ctx.enter_context(tc.tile_pool(name="qkv", bufs=1))
        ppool = ctx.enter_context(tc.tile_pool(name="pp", bufs=10))
        npool = ctx.enter_context(tc.tile_pool(name="np", bufs=2))
        opool = ctx.enter_context(tc.tile_pool(name="op", bufs=4))
        pss = ctx.enter_context(tc.tile_pool(name="pss", bufs=3, space="PSUM"))
        psw = ctx.enter_context(tc.tile_pool(name="psw", bufs=1, space="PSUM"))
        psb = ctx.enter_context(tc.tile_pool(name="psb", bufs=2, space="PSUM"))

        # constants (DMAs split across the two HWDGE engines: SP + ACT)
        ones = consts.tile([128, 32], F16, tag="ones")
        nc.vector.memset(ones, 1.0)
        e_sb = consts.tile([97, 256], F16, tag="E")
        nc.vector.memset(e_sb, 0.0)
        for pair in range(2):
            # head 2p sums live at psum row 64p, head 2p+1 at row 64p+32
            nc.vector.memset(
                e_sb[64 * pair : 64 * pair + 1, 128 * pair : 128 * pair + 64], 1.0 / 64
            )
            nc.vector.memset(
                e_sb[64 * pair + 32 : 64 * pair + 33, 128 * pair + 64 : 128 * pair + 128],
                1.0 / 64,
            )

        bq_sb = consts.tile([128, 2], F32, tag="bq")
        mb_sb = consts.tile([128, kt_tiles], F32, tag="mb")
        nc.scalar.dma_start(out=mb_sb, in_=mb_d[:, :])
        nc.scalar.dma_start(out=bq_sb, in_=bq_d[:, :])

        wk_sb = wpool.tile([128, 8, CD], F16, tag="wk")
        wv_sb = wpool.tile([128, 8, CD], F16, tag="wv")
        wq_sb = wpool.tile([128, 8, CD], F16, tag="wq")
        wo_sb = wpool.tile([128, 2, D], F16, tag="wo")
        # memT/xT as half-tiles so projections start on half-arrived data
        memT_h = [
            bigin.tile([128, 4, k_pad], F16, tag=f"memT{h}", name=f"memT{h}")
            for h in range(2)
        ]
        xT_h = [
            bigin.tile([128, 4, S], F16, tag=f"xT{h}", name=f"xT{h}")
            for h in range(2)
        ]
        memT_r = memT.rearrange("(t p) s -> p t s", p=128)
        xT_r = xT.rearrange("(t p) s -> p t s", p=128)

        nc.sync.dma_start(out=wk_sb, in_=wk_d.rearrange("(t p) c -> p t c", p=128))
        nc.scalar.dma_start(out=wv_sb, in_=wv_d.rearrange("(t p) c -> p t c", p=128))
        for h in range(2):
            nc.sync.dma_start(
                out=memT_h[h][:, 0:2, :], in_=memT_r[:, 4 * h : 4 * h + 2, :]
            )
            nc.scalar.dma_start(
                out=memT_h[h][:, 2:4, :], in_=memT_r[:, 4 * h + 2 : 4 * h + 4, :]
            )
        nc.scalar.dma_start(out=wq_sb, in_=wq_d.rearrange("(t p) c -> p t c", p=128))
        for h in range(2):
            nc.sync.dma_start(
                out=xT_h[h][:, 0:2, :], in_=xT_r[:, 4 * h : 4 * h + 2, :]
            )
            nc.scalar.dma_start(
                out=xT_h[h][:, 2:4, :], in_=xT_r[:, 4 * h + 2 : 4 * h + 4, :]
            )
        nc.sync.dma_start(out=wo_sb, in_=wo_d.rearrange("(t p) c -> p t c", p=128))

        def memT_at(dt):
            return memT_h[dt // 4][:, dt % 4, :]

        def xT_at(dt):
            return xT_h[dt // 4][:, dt % 4, :]

        # ---- K/V projections (evacs on DVE) ---------------------------------
        QT = [
            [
                qkv.tile([128, 512], F16, tag=f"QT{p}_{q}", name=f"QT{p}_{q}")
                for q in range(4)
            ]
            for p in range(2)
        ]
        KT = [qkv.tile([128, k_pad], F16, tag=f"KT{p}", name=f"KT{p}") for p in range(2)]
        V = [qkv.tile([128, CD], F16, tag=f"V{kt}", name=f"V{kt}") for kt in range(kt_tiles)]

        for pair in range(2):
            cs = slice(128 * pair, 128 * pair + 128)
            for off, csz in kchunks:
                k_ps = pss.tile([128, 512], F32, tag="att", name="k_ps")
                for dt in range(8):
                    nc.tensor.matmul(
                        k_ps[:, 0:csz],
                        wk_sb[:, dt, cs],
                        memT_at(dt)[:, off : off + csz],
                        start=(dt == 0),
                        stop=(dt == 7),
                    )
                nc.vector.tensor_copy(KT[pair][:, off : off + csz], k_ps[:, 0:csz])

        for kt in range(kt_tiles):
            v_ps = pss.tile([128, CD], F32, tag="att", name="v_ps")
            for dt in range(8):
                nc.tensor.matmul(
                    v_ps,
                    memT_at(dt)[:, kt * 128 : (kt + 1) * 128],
                    wv_sb[:, dt, :],
                    start=(dt == 0),
                    stop=(dt == 7),
                )
            nc.vector.tensor_copy(V[kt], v_ps)

        def q_proj_one(qc, pair, tag, pool):
            cs = slice(128 * pair, 128 * pair + 128)
            q_ps = pool.tile([128, 512], F32, tag=tag, name="q_ps")
            for dt in range(8):
                nc.tensor.matmul(
                    q_ps,
                    wq_sb[:, dt, cs],
                    xT_at(dt)[:, qc * 512 : (qc + 1) * 512],
                    start=(dt == 0),
                    stop=(dt == 7),
                )
            nc.vector.tensor_scalar_add(
                QT[pair][qc], q_ps, bq_sb[:, pair : pair + 1]
            )

        for pair in range(2):
            q_proj_one(0, pair, "att", pss)

        def q_stream_gen():
            # Q1..Q3 through the "work" psum slot; fine-grained 4-dt steps
            for qqc in range(1, 4):
                for pair in range(2):
                    cs = slice(128 * pair, 128 * pair + 128)
                    q_ps = psw.tile([128, 512], F32, tag="work", name="q_ps")
                    for dt in range(8):
                        nc.tensor.matmul(
                            q_ps,
                            wq_sb[:, dt, cs],
                            xT_at(dt)[:, qqc * 512 : (qqc + 1) * 512],
                            start=(dt == 0),
                            stop=(dt == 7),
                        )
                        if dt == 3:
                            yield
                    nc.vector.tensor_scalar_add(
                        QT[pair][qqc], q_ps, bq_sb[:, pair : pair + 1]
                    )
                    yield

        # ---- attention: one flat software-pipelined stream ------------------
        outT = [
            [
                qkv.tile([128, 512], F16, tag=f"oT{p}_{q}", name=f"oT{p}_{q}")
                for q in range(4)
            ]
            for p in range(2)
        ]
        avp = ctx.enter_context(tc.tile_pool(name="avp", bufs=6))

        def norm_wo_half(qc, av_sb, recip_s, half, pool, tag):
            hs = slice(256 * half, 256 * half + 256)
            bc_sbs = []
            for pair in range(2):
                bc = pool.tile([128, 512], F32, tag=tag, name="bc")
                nc.tensor.matmul(
                    bc[:, 0:256],
                    e_sb[:, 128 * pair : 128 * pair + 128],
                    recip_s[:, 0:256],
                    start=True, stop=True,
                )
                bc_sb = npool.tile([128, 256], F32, tag="bc_sb", bufs=4)
                nc.vector.tensor_copy(bc_sb, bc[:, 0:256])
                bc_sbs.append(bc_sb)
                yield
            for pair in range(2):
                nc.vector.tensor_mul(
                    outT[pair][qc][:, hs], av_sb[pair][:, hs], bc_sbs[pair]
                )
                yield
            for sl in (2 * half, 2 * half + 1):
                o_sb = opool.tile([128, D], F32, tag="osb")
                for nch in range(2):
                    o_ps = pool.tile([128, 512], F32, tag=tag, name="o_ps")
                    for ct in range(2):
                        nc.tensor.matmul(
                            o_ps,
                            outT[ct][qc][:, sl * 128 : (sl + 1) * 128],
                            wo_sb[:, ct, nch * 512 : (nch + 1) * 512],
                            start=(ct == 0),
                            stop=(ct == 1),
                        )
                    if qc == 3 and nch == 1:
                        nc.scalar.copy(o_sb[:, nch * 512 : (nch + 1) * 512], o_ps)
                    else:
                        nc.vector.tensor_copy(
                            o_sb[:, nch * 512 : (nch + 1) * 512], o_ps
                        )
                    yield
                st = qc * 4 + sl
                eng = nc.scalar if (qc == 3 and sl % 2 == 1) else nc.sync
                eng.dma_start(out=out_d[st * 128 : (st + 1) * 128, :], in_=o_sb)

        # per-qc state, filled lazily while the flat stream runs
        st_av = {}      # qc -> [av0, av1] psum tiles
        st_sums = {}    # qc -> sums psum tile

        def flush(pend):
            qc, kt, pair, p_t = pend
            first = kt == 0
            last = kt == kt_tiles - 1
            av = st_av.setdefault(qc, [None, None])
            if av[pair] is None:
                av[pair] = pss.tile([128, 512], F32, tag="att", name=f"av{pair}")
            nc.tensor.matmul(
                av[pair][0:64, :],
                V[kt][:, 128 * pair : 128 * pair + 64],
                p_t[:, 0:512],
                start=first, stop=last,
            )
            nc.tensor.matmul(
                av[pair][64:128, :],
                V[kt][:, 128 * pair + 64 : 128 * pair + 128],
                p_t[:, 512:1024],
                start=first, stop=last,
            )
            if qc not in st_sums:
                st_sums[qc] = pss.tile([128, 512], F32, tag="att", name="sums_ps")
            sums_ps = st_sums[qc]
            nc.tensor.matmul(
                sums_ps[64 * pair : 64 * pair + 32, :],
                ones,
                p_t[:, 0:512],
                start=first, stop=last,
                tile_position=(0, 64 * pair),
            )
            nc.tensor.matmul(
                sums_ps[64 * pair + 32 : 64 * pair + 64, :],
                ones,
                p_t[:, 512:1024],
                start=first, stop=last,
                tile_position=(0, 64 * pair + 32),
            )

        def epilogue_steps(qc):
            """Evacuate av/sums psum, then per-half: 64/sums -> norm -> wo."""
            pool, tag = (pss, "att") if qc == 3 else (psw, "work")
            av = st_av[qc]
            av_sb = []
            for pair in range(2):
                t = avp.tile([128, 512], F32, tag="av_sb", name="av_sb")
                nc.vector.tensor_copy(t, av[pair])
                av_sb.append(t)
            yield
            for half in range(2):
                hs = slice(256 * half, 256 * half + 256)
                sums_sb = npool.tile([97, 256], F32, tag="sums_sb", bufs=4)
                nc.vector.tensor_scalar_mul(
                    sums_sb, st_sums[qc][0:97, hs], 1.0 / 64
                )
                yield
                recip32 = npool.tile([97, 256], F32, tag="recip32", bufs=4)
                nc.vector.reciprocal(out=recip32, in_=sums_sb)
                yield
                recip_s = npool.tile([97, 256], F16, tag="recip_s", bufs=4)
                nc.vector.tensor_copy(recip_s, recip32)
                yield
                # headroom for the DVE reciprocal before PE meets bc
                yield
                yield from norm_wo_half(qc, av_sb, recip_s, half, pool, tag)

        UNITS_PER_QC = kt_tiles * 2
        units = [
            (qc, kt, pair)
            for qc in range(4)
            for kt in range(kt_tiles)
            for pair in range(2)
        ]
        attach = {}
        for qc in range(4):
            attach[(qc + 1) * UNITS_PER_QC + 1] = qc

        prio = []
        qgen = q_stream_gen()
        pending = None
        for u, (qc, kt, pair) in enumerate(units):
            if qgen is not None and u % 2 == 0:
                if next(qgen, StopIteration) is StopIteration:
                    qgen = None
            if u in attach:
                prio.append(epilogue_steps(attach.pop(u)))
            ks = slice(kt * 128, (kt + 1) * 128)
            lt = psb.tile([128, 1024], F32, tag="lt")
            nc.tensor.matmul(
                lt[:, 0:512], KT[pair][0:64, ks], QT[pair][qc][0:64, :],
                start=True, stop=True,
            )
            nc.tensor.matmul(
                lt[:, 512:1024], KT[pair][64:128, ks], QT[pair][qc][64:128, :],
                start=True, stop=True,
            )
            p_t = ppool.tile([128, 1024], F16, tag="p")
            nc.scalar.activation(
                p_t, lt[:, :], Exp, bias=mb_sb[:, kt : kt + 1], scale=0.125
            )
            prio = [g for g in prio if next(g, StopIteration) is not StopIteration]
            if pending is not None:
                flush(pending)
            pending = (qc, kt, pair, p_t)

        flush(pending)
        if qgen is not None:
            for _ in qgen:
                pass
        # drain: leftover generators, then epilogue(3)
        for g in prio:
            for _ in g:
                pass
        for u in sorted(attach):
            for _ in epilogue_steps(attach[u]):
                pass

    return nc


_PROG_CACHE: dict[int, bass.Bass] = {}


def _get_prog(k_pad: int) -> bass.Bass:
    if k_pad not in _PROG_CACHE:
        _PROG_CACHE[k_pad] = _build(k_pad)
    return _PROG_CACHE[k_pad]


def _prep_inputs(x, memory, mask, wq, bq, wk, wv, k_pad):
    """Build the 8 per-core input maps."""
    kt_tiles = k_pad // 128
    in_maps = []
    per_batch = []
    for b in range(B):
        idx = np.flatnonzero(~mask[b])
        n = len(idx)
        assert n <= k_pad
        mem_g = np.zeros((k_pad, D), np.float16)
        mem_g[:n] = memory[b][idx].astype(np.float16)
        memT_b = np.ascontiguousarray(mem_g.T)
        xT_b = np.ascontiguousarray(x[b].astype(np.float16).T)
        mbias = np.zeros(k_pad, np.float32)
        mbias[n:] = NEG
        mb_b = np.ascontiguousarray(mbias.reshape(kt_tiles, 128).T)
        per_batch.append((xT_b, memT_b, mb_b, idx))
    for c in range(N_CORES):
        b, g = divmod(c, G)
        xT_b, memT_b, mb_b, _ = per_batch[b]
        cs = slice(g * CD, (g + 1) * CD)
        in_maps.append(
            {
                "xT": xT_b,
                "memT": memT_b,
                "wq": np.ascontiguousarray(wq[:, cs].astype(np.float16)),
                "wk": np.ascontiguousarray(wk[:, cs].astype(np.float16)),
                "wv": np.ascontiguousarray(wv[:, cs].astype(np.float16)),
                "bq": np.ascontiguousarray(bq[cs].reshape(2, 128).T.astype(np.float32)),
                "wo": None,  # filled by caller (needs wo)
                "maskb": mb_b,
            }
        )
    return in_maps, per_batch


def kernel(x, memory, mask, wq, bq, wk, bk, wv, bv, wo, bo, _trace=False):
    x = np.asarray(x, np.float32)
    memory = np.asarray(memory, np.float32)
    mask = np.asarray(mask).astype(bool)
    wq = np.asarray(wq, np.float32)
    bq = np.asarray(bq, np.float32)
    wk = np.asarray(wk, np.float32)
    wv = np.asarray(wv, np.float32)
    bv = np.asarray(bv, np.float32)
    wo = np.asarray(wo, np.float32)
    bo = np.asarray(bo, np.float32)

    nmax = max(int((~mask[b]).sum()) for b in range(B))
    k_pad = next(k for k in K_PAD_LADDER if k >= nmax)
    prog = _get_prog(k_pad)

    in_maps, _ = _prep_inputs(x, memory, mask, wq, bq, wk, wv, k_pad)
    for c in range(N_CORES):
        g = c % G
        in_maps[c]["wo"] = np.ascontiguousarray(
            wo[g * CD : (g + 1) * CD, :].astype(np.float16)
        )

    res = run_bass_kernel_spmd(prog, in_maps, list(range(N_CORES)), trace=_trace)
    outs = [res.results[c]["out"] for c in range(N_CORES)]
    final = np.empty((B, S, D), np.float32)
    tail = bo + bv @ wo
    for b in range(B):
        final[b] = outs[G * b]
        for g in range(1, G):
            final[b] += outs[G * b + g]
        final[b] += tail[None, :]
    if _trace:
        kernel.last_exec_time_ns = res.exec_time_ns
    return final

